# revision 1
# baseline (speedup 1.0000x reference)
"""Multi-head attention (B=4, S=2048, D=1024, H=16, dk=64) on 8 TRN2 cores.

Sharding: data-parallel over B (4 batches) x tensor-parallel over head
groups (2 groups of 8 heads).  Core c handles batch c//2 and head group
c%2: it computes Q/K/V with the 512-column slice of the projection
weights, runs attention for its 8 heads, and produces a partial output
projection through the matching 512-row slice of W_o.  The host sums the
two partials per batch and adds the constant bias term (bv @ Wo^T + bo).

Per-core kernel layout notes:
  - x is fed pre-transposed (xT [D, S]) so the contraction dim d lands on
    SBUF partitions for the Q/K projections.
  - Q^T, K^T are built in [e, s] layout (e on partitions) so scores can
    be computed transposed: S^T[k, q] = (K_h^T)^T-stationary @ Q_h^T.
    Softmax runs without max subtraction (scores are O(1) here), with the
    denominator obtained by appending a ones column to V in the
    attn@V matmul, and the division applied via a rank-1 broadcast
    (ones-outer-product matmul) + elementwise multiply.
  - All matmuls run as float32r (full-rate fp32 mode, N=512 >= 256).
"""

import sys

for _p in ("/opt/trn_rl_repo",):
    if _p not in sys.path:
        sys.path.insert(0, _p)

import numpy as np
from contextlib import ExitStack

import concourse.bass as bass
import concourse.mybir as mybir
import concourse.tile as tile
from concourse import bacc
from concourse.bass_utils import run_bass_kernel_spmd

F32 = mybir.dt.float32
F32R = mybir.dt.float32r
AF = mybir.ActivationFunctionType

D, S = 1024, 2048   # d_model, seq len
E = 512             # per-core projection width (8 heads x 64)
H, DK = 8, 64       # heads per core, head dim
NB = D // 128       # contraction chunks (8)
SCALE = 0.125       # 1/sqrt(dk)


def build_bass(n_attn_et=4, do_yproj=True):
    nc = bacc.Bacc(
        "TRN2", target_bir_lowering=False, debug=False, num_devices=8
    )
    xT = nc.dram_tensor("xT", [D, S], F32R, kind="ExternalInput").ap()
    xTf = nc.dram_tensor("xTf", [D, S], F32, kind="ExternalInput").ap()
    wq = nc.dram_tensor("wq", [D, E], F32, kind="ExternalInput").ap()
    wk = nc.dram_tensor("wk", [D, E], F32, kind="ExternalInput").ap()
    wv = nc.dram_tensor("wv", [D, E], F32R, kind="ExternalInput").ap()
    wo = nc.dram_tensor("wo", [E, D], F32R, kind="ExternalInput").ap()
    bq = nc.dram_tensor("bq", [E], F32, kind="ExternalInput").ap()
    bk = nc.dram_tensor("bk", [E], F32, kind="ExternalInput").ap()
    y = nc.dram_tensor("y", [S, D], F32, kind="ExternalOutput").ap()

    with ExitStack() as ctx:
        tc = ctx.enter_context(tile.TileContext(nc))
        const = ctx.enter_context(tc.tile_pool(name="const", bufs=1))
        wpool = ctx.enter_context(tc.tile_pool(name="wpool", bufs=8))
        xpool = ctx.enter_context(tc.tile_pool(name="xpool", bufs=12))
        qkpool = ctx.enter_context(tc.tile_pool(name="qkpool", bufs=2))
        res = ctx.enter_context(tc.tile_pool(name="res", bufs=1))
        epool = ctx.enter_context(tc.tile_pool(name="epool", bufs=4))
        ypool = ctx.enter_context(tc.tile_pool(name="ypool", bufs=2))
        bcpool = ctx.enter_context(tc.tile_pool(name="bcpool", bufs=2))
        rpool = ctx.enter_context(tc.tile_pool(name="rpool", bufs=2))
        ps_s = ctx.enter_context(tc.tile_pool(name="ps_s", bufs=2, space="PSUM"))
        ps_o = ctx.enter_context(tc.tile_pool(name="ps_o", bufs=4, space="PSUM"))

        # ---- constants ----
        bq_t = const.tile([128, 4], F32, tag="bq", name="bq_t")
        bk_t = const.tile([128, 4], F32, tag="bk", name="bk_t")
        ones_f = const.tile([128, 64], F32, tag="ones_f", name="ones_f")
        zeros_f = const.tile([128, 512], F32, tag="zf", name="zeros_f")
        ones = const.tile([1, 64], F32R, tag="ones", name="ones_t")
        nc.sync.dma_start(bq_t[:, :], bq.rearrange("(j p) -> p j", p=128))
        nc.sync.dma_start(bk_t[:, :], bk.rearrange("(j p) -> p j", p=128))
        nc.vector.memset(ones_f[:, :], 1.0)
        nc.vector.memset(zeros_f[:, :], 0.0)
        nc.scalar.copy(ones[:, :], ones_f[0:1, :])

        # ---- residents: V (with interleaved ones cols) and attn-out^T ----
        vt = [
            res.tile([128, H, 65], F32R, tag="vt", bufs=16, name=f"vt{i}")
            for i in range(16)
        ]
        ao = [
            res.tile([128, S], F32R, tag="ao", bufs=4, name=f"ao{i}")
            for i in range(4)
        ]

        def load_x_stripe(sc, label, dt_, src_):
            xs = []
            for dc in range(NB):
                xt_ = xpool.tile(
                    [128, 512], dt_, tag="xs", name=f"x_{label}_{sc}_{dc}"
                )
                nc.sync.dma_start(
                    xt_[:, :],
                    src_[dc * 128 : (dc + 1) * 128, sc * 512 : (sc + 1) * 512],
                )
                xs.append(xt_)
            return xs

        # ---- V projection (x stationary, Wv moving) ----
        wv_t = []
        for dc in range(NB):
            wvt = wpool.tile([128, 512], F32R, tag="w", name=f"wv{dc}")
            nc.sync.dma_start(wvt[:, :], wv[dc * 128 : (dc + 1) * 128, :])
            wv_t.append(wvt)
        for sc in range(4):
            xs = load_x_stripe(sc, "v", F32R, xT)
            for st in range(4):
                s_abs = sc * 4 + st
                vp = ps_s.tile([128, 512], F32, tag="s", name=f"vp{s_abs}")
                for dc in range(NB):
                    nc.tensor.matmul(
                        vp[:, :],
                        (xs[dc][:, st * 128 : (st + 1) * 128]),
                        (wv_t[dc][:, :]),
                        start=(dc == 0),
                        stop=(dc == NB - 1),
                    )
                nc.vector.tensor_copy(
                    vt[s_abs][:, :, 0:64], vp.rearrange("p (h d) -> p h d", h=H)
                )
                nc.vector.tensor_copy(
                    vt[s_abs][:, :, 64:65],
                    ones_f[:, 0:8].rearrange("p (h o) -> p h o", o=1),
                )

        # ---- per head-group-of-2 (one e-tile): Q/K projection + attention ----
        for et in range(4):
            wq_t = wpool.tile([128, NB, 128], F32, tag="w", name=f"wq{et}")
            nc.sync.dma_start(
                wq_t[:, :, :],
                wq.rearrange("(dc p) e -> p dc e", p=128)[
                    :, :, et * 128 : (et + 1) * 128
                ],
            )
            wk_t = wpool.tile([128, NB, 128], F32, tag="w", name=f"wk{et}")
            nc.sync.dma_start(
                wk_t[:, :, :],
                wk.rearrange("(dc p) e -> p dc e", p=128)[
                    :, :, et * 128 : (et + 1) * 128
                ],
            )
            qT_t = qkpool.tile([128, S], F32R, tag="qT", name=f"qT{et}")
            kp = [
                qkpool.tile([128, S], F32R, tag="kp", bufs=2, name=f"kp{et}_{j}")
                for j in range(2)
            ]
            for sc in range(4):
                xs = load_x_stripe(sc, f"qk{et}", F32, xTf)
                sl_ = slice(sc * 512, (sc + 1) * 512)
                for wt_, is_k in ((wq_t, False), (wk_t, True)):
                    pp = ps_s.tile([128, 512], F32, tag="s", name=f"pp{et}_{sc}")
                    for dc in range(NB):
                        nc.tensor.matmul(
                            pp[:, :],
                            (wt_[:, dc, :]),
                            (xs[dc][:, :]),
                            start=(dc == 0),
                            stop=(dc == NB - 1),
                        )
                    if not is_k:
                        nc.scalar.add(qT_t[:, sl_], pp[:, :], bq_t[:, et : et + 1])
                    else:
                        # split K by head into zero-padded stationary tiles so
                        # scores matmuls run full-height (K=128 keeps HAM warm)
                        for j in range(2):
                            # head j's K occupies the same partition range as
                            # its Q rows (j*64..j*64+63); the other half is 0
                            nc.scalar.add(
                                kp[j][j * 64 : j * 64 + 64, sl_],
                                pp[j * 64 : j * 64 + 64, :],
                                bk_t[j * 64 : j * 64 + 64, et : et + 1],
                            )
                            nc.vector.tensor_copy(
                                kp[j][(1 - j) * 64 : (1 - j) * 64 + 64, sl_],
                                zeros_f[0:64, :],
                            )

            for hh in range(2 if et < n_attn_et else 0):
                h = 2 * et + hh
                off = hh * 64
                o_ps = [
                    ps_o.tile([65, 512], F32, tag="o", name=f"o{h}_{qc}")
                    for qc in range(4)
                ]
                prev_eps = None
                for kt in range(17):
                    eps = []
                    if kt < 16:
                        for pr in range(2):
                            sp = ps_s.tile(
                                [128, 1024], F32, tag="s", name=f"sp{h}_{kt}_{pr}"
                            )
                            for half in range(2):
                                qc = 2 * pr + half
                                nc.tensor.matmul(
                                    sp[:, half * 512 : (half + 1) * 512],
                                    (kp[hh][:, kt * 128 : (kt + 1) * 128]),
                                    (qT_t[:, qc * 512 : (qc + 1) * 512]),
                                    start=True,
                                    stop=True,
                                )
                            ep = epool.tile(
                                [128, 1024], F32R, tag="e", name=f"ep{h}_{kt}_{pr}"
                            )
                            nc.scalar.activation(ep[:, :], sp[:, :], AF.Exp, scale=SCALE)
                            eps.append(ep)
                    if prev_eps is not None:
                        pk = kt - 1
                        for qc in range(4):
                            nc.tensor.matmul(
                                o_ps[qc][:, :],
                                (vt[pk][:, h, :]),
                                (prev_eps[qc // 2][:, (qc % 2) * 512 : (qc % 2 + 1) * 512]),
                                start=(pk == 0),
                                stop=(pk == 15),
                            )
                    prev_eps = eps if kt < 16 else None
                for qc in range(4):
                    recip = rpool.tile([1, 512], F32, tag="r", name=f"rc{h}_{qc}")
                    nc.vector.reciprocal(recip[:, :], o_ps[qc][64:65, :])
                    # broadcast 1/denom to 64 partitions on the idle GpSimd —
                    # no PE matmul, no PSUM slot, no ACT copy
                    bc_sb = bcpool.tile([64, 512], F32, tag="bc", name=f"bs{h}_{qc}")
                    nc.gpsimd.partition_broadcast(bc_sb[:, :], recip[:, :])
                    nc.vector.tensor_mul(
                        ao[et][off : off + 64, qc * 512 : (qc + 1) * 512],
                        o_ps[qc][0:64, :],
                        bc_sb[:, :],
                    )

        # ---- output projection (partial: this core's 512 e-rows of Wo) ----
        wo_t = []
        for ec in range(4):
            wot = wpool.tile([128, 1024], F32R, tag="w", name=f"wo{ec}")
            nc.sync.dma_start(wot[:, :], wo[ec * 128 : (ec + 1) * 128, :])
            wo_t.append(wot)
        for qt in range(16 if do_yproj else 0):
            yps = [
                ps_s.tile([128, 512], F32, tag="s", name=f"yp{qt}_{oc}")
                for oc in range(2)
            ]
            for ec in range(4):
                for oc in range(2):
                    nc.tensor.matmul(
                        yps[oc][:, :],
                        (ao[ec][:, qt * 128 : (qt + 1) * 128]),
                        (wo_t[ec][:, oc * 512 : (oc + 1) * 512]),
                        start=(ec == 0),
                        stop=(ec == 3),
                    )
            ysb = ypool.tile([128, 1024], F32, tag="y", name=f"ysb{qt}")
            for oc in range(2):
                nc.vector.tensor_copy(ysb[:, oc * 512 : (oc + 1) * 512], yps[oc][:, :])
            nc.sync.dma_start(y[qt * 128 : (qt + 1) * 128, :], ysb[:, :])

    nc.finalize()
    return nc


def make_in_maps(x, Wq, Wk, Wv, Wo, bq, bk):
    in_maps = []
    for c in range(8):
        b, g = divmod(c, 2)
        sl = slice(g * E, (g + 1) * E)
        in_maps.append(
            {
                "xT": np.ascontiguousarray(x[b].T, dtype=np.float32),
                "xTf": np.ascontiguousarray(x[b].T, dtype=np.float32),
                "wq": np.ascontiguousarray(Wq[sl, :].T, dtype=np.float32),
                "wk": np.ascontiguousarray(Wk[sl, :].T, dtype=np.float32),
                "wv": np.ascontiguousarray(Wv[sl, :].T, dtype=np.float32),
                "wo": np.ascontiguousarray(Wo[:, sl].T, dtype=np.float32),
                "bq": np.ascontiguousarray(bq[sl], dtype=np.float32),
                "bk": np.ascontiguousarray(bk[sl], dtype=np.float32),
            }
        )
    return in_maps


_NC = None


def run(x, Wq, bq, Wk, bk, Wv, bv, Wo, bo, build_kwargs=None, **run_kwargs):
    global _NC
    x = np.asarray(x, dtype=np.float32)
    Wq, Wk, Wv, Wo = (np.asarray(a, dtype=np.float32) for a in (Wq, Wk, Wv, Wo))
    bq, bk, bv, bo = (np.asarray(a, dtype=np.float32) for a in (bq, bk, bv, bo))
    if _NC is None:
        _NC = build_bass(**(build_kwargs or {}))
    in_maps = make_in_maps(x, Wq, Wk, Wv, Wo, bq, bk)
    try:
        res = run_bass_kernel_spmd(
            _NC, in_maps, core_ids=list(range(8)), **run_kwargs
        )
    except Exception:
        # One retry: a previously wedged device can fail the first attempt.
        res = run_bass_kernel_spmd(
            _NC, in_maps, core_ids=list(range(8)), **run_kwargs
        )
    ys = [r["y"] for r in res.results]
    c_vec = (bv @ Wo.T + bo).astype(np.float32)  # constant bias fold
    out = np.stack([ys[2 * b] + ys[2 * b + 1] + c_vec for b in range(4)])
    return out.astype(np.float32), res


def kernel(x, Wq, bq, Wk, bk, Wv, bv, Wo, bo):
    out, _ = run(x, Wq, bq, Wk, bk, Wv, bv, Wo, bo)
    return out



# revision 6
# speedup vs baseline: 1.5764x; 1.5764x over previous
"""Multi-head attention (B=4, S=2048, D=1024, H=16, dk=64) on 8 TRN2 cores.

Sharding: data-parallel over B (4 batches) x tensor-parallel over head
groups (2 groups of 8 heads).  Core c handles batch c//2 and head group
c%2: it computes Q/K/V with the 512-column slice of the projection
weights, runs attention for its 8 heads, and produces a partial output
projection through the matching 512-row slice of W_o.  The host sums the
two partials per batch and adds the constant bias term (bv @ Wo^T + bo).

v2 kernel layout:
  - All matmul operands are fp16 (full-rate, FWL weight loads overlap the
    previous matmul) except the attn@V pass which runs fp8e4m3 DoubleRow
    (2 k-position chunks per instruction).  PSUM accumulation is fp32.
  - x^T and all weights are pre-laid-out on the host ([p, chunk, free])
    and DMA'd once into SBUF residents.
  - K bias is dropped entirely (softmax is invariant to per-query score
    shifts); Q bias is applied by the DVE during the PSUM->SBUF copy.
  - Scores for the two heads of an e-tile run as K=64 matmuls on PE row
    groups 0-63 / 64-127 concurrently (tile_position row tiling).
  - exp() on ACT writes fp8 eps directly; softmax denominator comes from
    a ones column appended to V; 1/denom via reciprocal_approx_fast.
"""

import sys

for _p in ("/opt/trn_rl_repo",):
    if _p not in sys.path:
        sys.path.insert(0, _p)

import numpy as np
from contextlib import ExitStack

import concourse.bass as bass
import concourse.mybir as mybir
import concourse.tile as tile
from concourse import bacc
from concourse.bass_utils import run_bass_kernel_spmd

F32 = mybir.dt.float32
F16 = mybir.dt.float16
F8 = mybir.dt.float8e4
AF = mybir.ActivationFunctionType
DR = mybir.MatmulPerfMode.DoubleRow

D, S = 1024, 2048   # d_model, seq len
E = 512             # per-core projection width (8 heads x 64)
H, DK = 8, 64       # heads per core, head dim
NB = D // 128       # contraction chunks (8)
NQT = S // 128      # out-proj q chunks (16)
SCALE = 0.125       # 1/sqrt(dk)


def build_bass(use_dr=True, pair_scores=True, fast_recip=True):
    nc = bacc.Bacc(
        "TRN2", target_bir_lowering=False, debug=False, num_devices=8
    )
    xd = nc.dram_tensor("xd", [128, NB, S], F16, kind="ExternalInput").ap()
    wqd = nc.dram_tensor("wqd", [128, NB, E], F16, kind="ExternalInput").ap()
    wkd = nc.dram_tensor("wkd", [128, NB, E], F16, kind="ExternalInput").ap()
    wvd = nc.dram_tensor("wvd", [128, NB, E], F16, kind="ExternalInput").ap()
    wod = nc.dram_tensor("wod", [128, 4, D], F16, kind="ExternalInput").ap()
    bqd = nc.dram_tensor("bqd", [128, 4], F32, kind="ExternalInput").ap()
    y = nc.dram_tensor("y", [S, D], F32, kind="ExternalOutput").ap()

    with ExitStack() as ctx:
        tc = ctx.enter_context(tile.TileContext(nc))
        const = ctx.enter_context(tc.tile_pool(name="const", bufs=1))
        res = ctx.enter_context(tc.tile_pool(name="res", bufs=1))
        qkpool = ctx.enter_context(tc.tile_pool(name="qkpool", bufs=2))
        epool = ctx.enter_context(tc.tile_pool(name="epool", bufs=3))
        drpool = ctx.enter_context(tc.tile_pool(name="drpool", bufs=2))
        bcpool = ctx.enter_context(tc.tile_pool(name="bcpool", bufs=2))
        ypool = ctx.enter_context(tc.tile_pool(name="ypool", bufs=2))
        ps_sp = ctx.enter_context(tc.tile_pool(name="ps_sp", bufs=2, space="PSUM"))
        ps_o = ctx.enter_context(tc.tile_pool(name="ps_o", bufs=2, space="PSUM"))
        ps_p = ctx.enter_context(tc.tile_pool(name="ps_p", bufs=2, space="PSUM"))

        # ---- residents ----
        xsb = res.tile([128, NB, S], F16, tag="x", name="xsb")
        wqt = res.tile([128, NB, E], F16, tag="wq", name="wqt")
        wkt = res.tile([128, NB, E], F16, tag="wk", name="wkt")
        wvt = res.tile([128, NB, E], F16, tag="wv", name="wvt")
        wot = res.tile([128, 4, D], F16, tag="wo", name="wot")
        bqt = res.tile([128, 4], F32, tag="bq", name="bqt")
        # V resident: per kt-pair, [p, pair, head, 80] fp8 (col 64 = ones,
        # cols 65-79 pad so the DoubleRow k-subtile step is 16B-aligned)
        vdt = F8 if use_dr else F16
        vt = [
            res.tile([128, 2, H, 80], vdt, tag="vt", bufs=8, name=f"vt{i}")
            for i in range(8)
        ]
        ao = [
            res.tile([128, S], F16, tag="ao", bufs=4, name=f"ao{i}")
            for i in range(4)
        ]

        for dc in range(NB):
            nc.sync.dma_start(xsb[:, dc, :], xd[:, dc, :])
        nc.sync.dma_start(wvt[:, :, :], wvd[:, :, :])
        nc.sync.dma_start(wqt[:, :, :], wqd[:, :, :])
        nc.sync.dma_start(wkt[:, :, :], wkd[:, :, :])
        nc.sync.dma_start(bqt[:, :], bqd[:, :])
        nc.sync.dma_start(wot[:, :, :], wod[:, :, :])
        for i in range(8):
            nc.vector.memset(vt[i][:, :, :, 64:65], 1.0)

        def qk_proj_chunk(et, sc):
            """Build qT/kT columns [sc*512, (sc+1)*512) for e-tile et."""
            sl_ = slice(sc * 512, (sc + 1) * 512)
            esl = slice(et * 128, (et + 1) * 128)
            for wt_, is_k in ((wqt, False), (wkt, True)):
                pp = ps_p.tile([128, 512], F32, tag="p", name=f"pp{et}_{sc}_{int(is_k)}")
                for dc in range(NB):
                    nc.tensor.matmul(
                        pp[:, :],
                        wt_[:, dc, esl],
                        xsb[:, dc, sl_],
                        start=(dc == 0),
                        stop=(dc == NB - 1),
                    )
                if is_k:
                    nc.vector.tensor_copy(kT[et % 2][:, sl_], pp[:, :])
                else:
                    nc.vector.tensor_scalar_add(
                        qT[et % 2][:, sl_], pp[:, :], bqt[:, et : et + 1]
                    )

        # qT/kT double-buffered across e-tiles
        qT = [qkpool.tile([128, S], F16, tag="qT", name=f"qT{j}") for j in range(2)]
        kT = [qkpool.tile([128, S], F16, tag="kT", name=f"kT{j}") for j in range(2)]

        # ---- Q/K projection for e-tile 0 first so ACT work starts early ----
        for sc in range(4):
            qk_proj_chunk(0, sc)

        # ---- V projection (x stationary, Wv moving) -> vt residents ----
        for tp in range(8):
            for par in range(2):
                s_abs = 2 * tp + par
                vp = ps_p.tile([128, 512], F32, tag="p", name=f"vp{s_abs}")
                for dc in range(NB):
                    nc.tensor.matmul(
                        vp[:, :],
                        xsb[:, dc, s_abs * 128 : (s_abs + 1) * 128],
                        wvt[:, dc, :],
                        start=(dc == 0),
                        stop=(dc == NB - 1),
                    )
                nc.vector.tensor_copy(
                    vt[tp][:, par, :, 0:64],
                    vp.rearrange("p (h d) -> p h d", h=H),
                )

        # ---- attention + interleaved next-et projection + out-proj ----
        for et in range(4):
            qTe, kTe = qT[et % 2], kT[et % 2]
            for qc in range(4):
                qsl = slice(qc * 512, (qc + 1) * 512)
                o_h = [
                    ps_o.tile([65, 512], F32, tag="o", name=f"o{et}_{qc}_{h}")
                    for h in range(2)
                ]
                ep = None
                for kt in range(16):
                    ksl = slice(kt * 128, (kt + 1) * 128)
                    sp = ps_sp.tile([128, 1024], F32, tag="sp", name=f"sp{et}_{qc}_{kt}")
                    if pair_scores:
                        nc.tensor.matmul(
                            sp[:, 0:512], kTe[0:64, ksl], qTe[0:64, qsl],
                            start=True, stop=True, tile_position=(0, 0),
                        )
                        nc.tensor.matmul(
                            sp[:, 512:1024], kTe[64:128, ksl], qTe[64:128, qsl],
                            start=True, stop=True, tile_position=(64, 0),
                        )
                    else:
                        for h in range(2):
                            nc.tensor.matmul(
                                sp[:, h * 512 : (h + 1) * 512],
                                kTe[h * 64 : h * 64 + 64, ksl],
                                qTe[h * 64 : h * 64 + 64, qsl],
                                start=True, stop=True,
                                tile_position=(64 * h, 0),
                            )
                    par = kt % 2
                    if par == 0:
                        ep = epool.tile(
                            [128, 2, 2, 512], F8 if use_dr else F16,
                            tag="eps", name=f"ep{et}_{qc}_{kt // 2}",
                        )
                    # one exp covers both heads; out strided [h, par, q]
                    nc.scalar.activation(
                        ep[:, :, par, :],
                        sp.rearrange("p (h q) -> p h q", h=2),
                        AF.Exp,
                        scale=SCALE,
                    )
                    if par == 1:
                        tp = kt // 2
                        for h in range(2):
                            hg = 2 * et + h
                            if use_dr:
                                nc.tensor.matmul(
                                    o_h[h][:, :],
                                    vt[tp][:, :, hg, 0:65],
                                    ep[:, h, :, :],
                                    start=(tp == 0),
                                    stop=(tp == 7),
                                    perf_mode=DR,
                                )
                            else:
                                for par2 in range(2):
                                    nc.tensor.matmul(
                                        o_h[h][:, :],
                                        vt[tp][:, par2, hg, 0:65],
                                        ep[:, h, par2, :],
                                        start=(tp == 0 and par2 == 0),
                                        stop=(tp == 7 and par2 == 1),
                                    )
                # normalize: ao[et][h*64:+64, qsl] = o_h[h][0:64] / denom
                for h in range(2):
                    dr_t = drpool.tile([1, 512], F32, tag="dr", name=f"dr{et}_{qc}_{h}")
                    if fast_recip:
                        # stage through SBUF: the custom-DVE recip misreads PSUM
                        dcp = drpool.tile(
                            [1, 512], F32, tag="dcp", name=f"dcp{et}_{qc}_{h}"
                        )
                        nc.vector.tensor_copy(dcp[:, :], o_h[h][64:65, :])
                        nc.vector.reciprocal_approx_fast(dr_t[:, :], dcp[:, :])
                    else:
                        nc.vector.reciprocal(dr_t[:, :], o_h[h][64:65, :])
                    bc_t = bcpool.tile([64, 512], F32, tag="bc", name=f"bc{et}_{qc}_{h}")
                    nc.gpsimd.partition_broadcast(bc_t[:, :], dr_t[:, :])
                    nc.vector.tensor_mul(
                        ao[et][h * 64 : (h + 1) * 64, qsl],
                        o_h[h][0:64, :],
                        bc_t[:, :],
                    )
                if et < 3:
                    # hide the next e-tile's Q/K projection in the ACT-bound
                    # attention stretch
                    qk_proj_chunk(et + 1, qc)
                else:
                    # out-projection for the q-rows this qc completed
                    for qt in range(4 * qc, 4 * qc + 4):
                        qtsl = slice(qt * 128, (qt + 1) * 128)
                        yps = [
                            ps_p.tile([128, 512], F32, tag="p", name=f"yp{qt}_{oc}")
                            for oc in range(2)
                        ]
                        for ec in range(4):
                            for oc in range(2):
                                nc.tensor.matmul(
                                    yps[oc][:, :],
                                    ao[ec][:, qtsl],
                                    wot[:, ec, oc * 512 : (oc + 1) * 512],
                                    start=(ec == 0),
                                    stop=(ec == 3),
                                )
                        ysb = ypool.tile([128, D], F32, tag="y", name=f"ysb{qt}")
                        for oc in range(2):
                            nc.vector.tensor_copy(
                                ysb[:, oc * 512 : (oc + 1) * 512], yps[oc][:, :]
                            )
                        nc.sync.dma_start(y[qtsl, :], ysb[:, :])

    nc.finalize()
    return nc


def make_in_maps(x, Wq, Wk, Wv, Wo, bq):
    def chunked(w):  # [D, n] -> [128, D//128, n]
        n = w.shape[1]
        return np.ascontiguousarray(
            w.reshape(-1, 128, n).transpose(1, 0, 2), dtype=np.float16
        )

    in_maps = []
    for c in range(8):
        b, g = divmod(c, 2)
        sl = slice(g * E, (g + 1) * E)
        in_maps.append(
            {
                "xd": chunked(x[b].T),                 # [128, 8, S]
                "wqd": chunked(Wq[sl, :].T),           # [128, 8, E]
                "wkd": chunked(Wk[sl, :].T),
                "wvd": chunked(Wv[sl, :].T),
                "wod": chunked(Wo[:, sl].T),           # [128, 4, D]
                "bqd": np.ascontiguousarray(
                    bq[sl].reshape(4, 128).T, dtype=np.float32
                ),
            }
        )
    return in_maps


_NC = None


def run(x, Wq, bq, Wk, bk, Wv, bv, Wo, bo, build_kwargs=None, **run_kwargs):
    global _NC
    x = np.asarray(x, dtype=np.float32)
    Wq, Wk, Wv, Wo = (np.asarray(a, dtype=np.float32) for a in (Wq, Wk, Wv, Wo))
    bq, bk, bv, bo = (np.asarray(a, dtype=np.float32) for a in (bq, bk, bv, bo))
    if _NC is None:
        _NC = build_bass(**(build_kwargs or {}))
    in_maps = make_in_maps(x, Wq, Wk, Wv, Wo, bq)
    try:
        res = run_bass_kernel_spmd(
            _NC, in_maps, core_ids=list(range(8)), **run_kwargs
        )
    except Exception:
        # One retry: a previously wedged device can fail the first attempt.
        res = run_bass_kernel_spmd(
            _NC, in_maps, core_ids=list(range(8)), **run_kwargs
        )
    ys = [r["y"] for r in res.results]
    c_vec = (bv @ Wo.T + bo).astype(np.float32)  # constant bias fold
    out = np.stack([ys[2 * b] + ys[2 * b + 1] + c_vec for b in range(4)])
    return out.astype(np.float32), res


def kernel(x, Wq, bq, Wk, bk, Wv, bv, Wo, bo):
    out, _ = run(x, Wq, bq, Wk, bk, Wv, bv, Wo, bo)
    return out
